# revision 13
# baseline (speedup 1.0000x reference)
"""Per-clause softmax-attention pooling over BERT hidden states on 8 trn2
NeuronCores (data-parallel over the batch dim).

Math (per batch b): s = H @ w5 (+b5); per contiguous clause segment j,
w = softmax(s[seg j]); out[j] = w @ H[seg j].  Softmax is shift-invariant so
b5 and the max-subtraction cancel exactly; scores are ~N(0, 0.55^2) so raw
exp() is numerically safe in fp32.

Device pipeline per batch (t on partitions, 4 chunks of 128 tokens):
  DVE  tensor_tensor_reduce: s[t] = sum_d H[t,d]*w5[d]   (fused mul+reduce)
  ACT  p = exp(s)
  DVE  Wt[t,j] = mask[t,j] * p[t]   (host-built one-hot segment mask)
  PE   out[32,768] += Wt.T @ H ; den[32,1] += Wt.T @ ones  (4 K-chunks)
  DVE  rden = 1/den ; ACT out_sbuf = psum * rden ; DMA out.
"""

import sys

for _p in ("/opt/trn_rl_repo", "/root/.axon_site/_ro/trn_rl_repo"):
    if _p not in sys.path:
        sys.path.insert(0, _p)

import numpy as np

import concourse.bass as bass
import concourse.bass2jax as bass2jax
import concourse.bass_utils as bass_utils
import concourse.tile as tile
from concourse import mybir
from concourse.bass_utils import run_bass_kernel_spmd

B, T, D, C = 64, 512, 768, 31
NSEG = C + 1  # 32 segments
N_CORES = 8
BPC = B // N_CORES  # batches per core
KCH = T // 128  # 4 token chunks of 128
F32 = mybir.dt.float32

# ---------------------------------------------------------------------------
# walrus in this container accepts at most ONE sync-wait command per
# instruction ("Too many sync wait commands"), but Tile emits several on
# fan-in instructions.  Rewrite the BIR before compiling: excess waits move
# onto standalone EventSemaphore carrier instructions inserted just before,
# on the same engine (same semantics — the engine executes them in order).
# ---------------------------------------------------------------------------
_MAX_WAITS_PER_INST = 1
_orig_compile_bir_kernel = bass_utils.compile_bir_kernel


def _split_bir_waits(bir_json: bytes) -> bytes:
    import orjson

    bir = orjson.loads(bir_json)
    ctr = 0
    for fn in bir.get("functions", []):
        for blk in fn.get("blocks", []):
            out_insts = []
            changed = False
            for inst in blk.get("instructions", []):
                si = inst.get("sync_info")
                waits = (si or {}).get("on_wait") or []
                if len(waits) > _MAX_WAITS_PER_INST:
                    changed = True
                    head = waits[: -_MAX_WAITS_PER_INST]
                    si["on_wait"] = waits[-_MAX_WAITS_PER_INST :]
                    for w in head:
                        ctr += 1
                        out_insts.append(
                            {
                                "engine": inst["engine"],
                                "ins": [],
                                "outs": [],
                                "name": f"antwaitsplit_{ctr}",
                                "opcode": "EventSemaphore",
                                "sync_info": {"on_update": [], "on_wait": [w]},
                            }
                        )
                out_insts.append(inst)
            if changed:
                blk["instructions"] = out_insts
    return orjson.dumps(bir)


def _patched_compile_bir_kernel(bir_json, tmpdir, neff_name="file.neff"):
    return _orig_compile_bir_kernel(_split_bir_waits(bir_json), tmpdir, neff_name)


bass_utils.compile_bir_kernel = _patched_compile_bir_kernel
bass2jax.compile_bir_kernel = _patched_compile_bir_kernel


def _build_nc(rep: int = 1):
    nc = bass.Bass()
    hs = nc.dram_tensor("hidden", [BPC, T, D], F32, kind="ExternalInput")
    # host-prebuilt one-hot segment masks, laid out [b, p, k, j]
    mk = nc.dram_tensor("mask", [BPC, 128, KCH, NSEG], F32, kind="ExternalInput")
    w5t = nc.dram_tensor("w5", [D], F32, kind="ExternalInput")
    out = nc.dram_tensor("out", [BPC, NSEG, D], F32, kind="ExternalOutput")

    with tile.TileContext(nc) as tc:
        with (
            tc.tile_pool(name="const", bufs=1) as const,
            tc.tile_pool(name="hpool", bufs=3) as hpool,
            tc.tile_pool(name="mpool", bufs=3) as mpool,
            tc.tile_pool(name="spool", bufs=4) as spool,
            tc.tile_pool(name="scratch", bufs=4) as scratch,
            tc.tile_pool(name="wpool", bufs=3) as wpool,
            tc.tile_pool(name="opool", bufs=3) as opool,
            tc.tile_pool(name="rpool", bufs=4) as rpool,
            tc.tile_pool(name="psum", bufs=2, space="PSUM") as psum,
        ):
            w5rep = const.tile([128, D], F32)
            nc.gpsimd.dma_start(
                out=w5rep,
                in_=bass.AP(tensor=w5t, offset=0, ap=[[0, 128], [1, D]]),
            )
            ones = const.tile([128, 1], F32)
            nc.vector.memset(ones, 1.0)

            for b in [b for _ in range(rep) for b in range(BPC)]:
                ht = hpool.tile([128, KCH, D], mybir.dt.float32r)
                hsb = hs[b].rearrange("(k p) d -> k p d", p=128)
                for k in range(KCH):
                    nc.sync.dma_start(out=ht[:, k], in_=hsb[k].bitcast(mybir.dt.float32r))
                mt = mpool.tile([128, KCH, NSEG], F32)
                nc.sync.dma_start(out=mt, in_=mk[b])

                s = spool.tile([128, KCH], F32)
                for k in range(KCH):
                    prod = scratch.tile([128, D], F32)
                    nc.vector.tensor_mul(prod, ht[:, k].bitcast(F32), w5rep)
                    junk = scratch.tile([128, D], mybir.dt.bfloat16, tag="actjunk")
                    nc.scalar.activation(
                        out=junk,
                        in_=prod,
                        func=mybir.ActivationFunctionType.Copy,
                        accum_out=s[:, k : k + 1],
                    )
                p = spool.tile([128, KCH], F32)
                nc.scalar.activation(
                    out=p, in_=s, func=mybir.ActivationFunctionType.Exp
                )
                wt = wpool.tile([128, KCH, NSEG], mybir.dt.float32r)
                for k in range(KCH):
                    nc.vector.tensor_scalar_mul(wt[:, k], mt[:, k], p[:, k : k + 1])

                po1 = psum.tile([NSEG, 384], F32)
                po2 = psum.tile([NSEG, 384], F32)
                pd = psum.tile([NSEG, 1], F32)
                F32R = mybir.dt.float32r
                for k in range(KCH):
                    st, sp = (k == 0), (k == KCH - 1)
                    wtk = wt[:, k]
                    nc.tensor.matmul(
                        pd, lhsT=wtk.bitcast(F32), rhs=ones, start=st, stop=sp
                    )
                    nc.tensor.matmul(
                        po1,
                        lhsT=wtk,
                        rhs=ht[:, k, 0:384],
                        start=st,
                        stop=sp,
                    )
                    nc.tensor.matmul(
                        po2,
                        lhsT=wtk,
                        rhs=ht[:, k, 384:768],
                        start=st,
                        stop=sp,
                    )

                rd = rpool.tile([NSEG, 1], F32)
                nc.vector.reciprocal(out=rd, in_=pd)
                ob = opool.tile([NSEG, D], F32)
                nc.scalar.activation(
                    out=ob[:, 0:384],
                    in_=po1,
                    func=mybir.ActivationFunctionType.Copy,
                    scale=rd,
                )
                nc.scalar.activation(
                    out=ob[:, 384:768],
                    in_=po2,
                    func=mybir.ActivationFunctionType.Copy,
                    scale=rd,
                )
                nc.sync.dma_start(out=out[b], in_=ob)
    return nc


def _segment_masks(clause_b: np.ndarray) -> np.ndarray:
    """One-hot [B, 128, KCH, NSEG] fp32 masks: token t = k*128+p is in
    segment j iff (# boundaries <= t) == j."""
    t_idx = np.arange(T)
    seg = (clause_b[:, None, :].astype(np.int64) <= t_idx[None, :, None]).sum(-1)
    onehot = (seg[..., None] == np.arange(NSEG)).astype(np.float32)  # [B, T, NSEG]
    return np.ascontiguousarray(
        onehot.reshape(B, KCH, 128, NSEG).transpose(0, 2, 1, 3)
    )


def kernel(hidden_state, clause_b, w5, b5):
    hidden_state = np.asarray(hidden_state, dtype=np.float32)
    clause_b = np.asarray(clause_b)
    w5 = np.asarray(w5, dtype=np.float32).reshape(D)
    masks = _segment_masks(np.asarray(clause_b, dtype=np.int32))

    nc = _build_nc()
    in_maps = []
    for i in range(N_CORES):
        sl = slice(i * BPC, (i + 1) * BPC)
        in_maps.append(
            {
                "hidden": np.ascontiguousarray(hidden_state[sl]),
                "mask": np.ascontiguousarray(masks[sl]),
                "w5": w5,
            }
        )
    res = run_bass_kernel_spmd(nc, in_maps, core_ids=list(range(N_CORES)))
    out = np.concatenate([res.results[i]["out"] for i in range(N_CORES)], axis=0)
    return out.astype(np.float32)


if __name__ == "__main__":
    rng = np.random.default_rng(0)
    hs = rng.standard_normal((B, T, D), dtype=np.float32)
    step = T // (C + 1)
    base = np.arange(1, C + 1) * step
    cb = (base[None, :] + rng.integers(0, step - 1, (B, C))).astype(np.int32)
    w5 = (rng.standard_normal((D, 1)) * 0.02).astype(np.float32)
    b5 = np.zeros(1, np.float32)
    got = kernel(hs, cb, w5, b5)
    print("kernel out:", got.shape, got.dtype)
